# revision 20
# baseline (speedup 1.0000x reference)
"""InternLM3 self-attention (prefill, GQA, RoPE) on 8 Trainium2 cores.

Tensor-parallel over heads: core r owns q heads 4r..4r+3 and kv head r
(wqkv column shards, wo row shards).  Each core computes its partial
output projection in bf16; the 8 partials are summed on the host.

v2.2 design (baseline fp32r v1 was ~640-700us):
  - every matmul in bf16 with 512-col moving operands (LDWEIGHTS mostly
    hidden behind the previous matmul; 1 cycle/row; fp32 PSUM).
  - softmax denominator for free: v rows carry an appended ones column,
    so the PV matmul accumulates sum(e) in PSUM column 128.
  - PV computed transposed (out [q, hd]) so 1/d normalization is a
    per-partition tensor_scalar with [128,1] reciprocals; the result
    moves into WO layout with a DMA-XBAR transpose (no PE).
  - v projected as v^T like q/k, then moved to natural [tok, hd] layout
    with DMA-XBAR transposes (no PE, no PSUM).
  - causal trimming at 128 granularity (diagonal 512-blocks stepped).
  - RoPE rotate-half via two partition-offset SBUF DMAs (no PE).
  - interleaved emission: the exp-gated score stream of each head is
    padded with a paced fill queue carrying the previous head's PV /
    normalize work and the previous block's WO groups, so the PE never
    waits on the scalar engine's exp stream.
  - PSUM discipline: concurrently-accumulating matmul groups sit in
    separate banks (start=True clears the whole bank's has_written).
"""

import collections

import numpy as np
from ml_dtypes import bfloat16

import concourse.bass as bass
import concourse.bacc as bacc
import concourse.mybir as mybir
import concourse.tile as tile
from concourse.bass_utils import run_bass_kernel_spmd

T = 2048
H = 4096
NH = 32
NKV = 8
HD = 128
HALF = HD // 2
BASE = 1000000.0
NCORES = 8
QH = NH // NCORES            # 4 q heads per core
QCOLS = QH * HD              # 512
NEG = -1e30

P = 128
G = 512                      # token chunk = attention q block
NG = T // G                  # 4
NHC = H // P                 # 32 contraction chunks
NKC = T // P                 # 16 k chunks of 128

f32 = mybir.dt.float32
bf16 = mybir.dt.bfloat16

_COMPILED = None
DEBUG_DUMP = False


def _build():
    nc = bacc.Bacc("TRN2", target_bir_lowering=False, debug=False,
                   num_devices=NCORES)

    hidT = nc.dram_tensor("hidT", [H, T], bf16, kind="ExternalInput").ap()
    wq_d = nc.dram_tensor("wq_d", [H, 768], bf16, kind="ExternalInput").ap()
    wo_d = nc.dram_tensor("wo_d", [P, QH, H], bf16,
                          kind="ExternalInput").ap()
    cosk = nc.dram_tensor("cosk", [P, T], bf16, kind="ExternalInput").ap()
    sink = nc.dram_tensor("sink", [P, T], bf16, kind="ExternalInput").ap()
    maskd = nc.dram_tensor("maskd", [P, P], f32, kind="ExternalInput").ap()
    part = nc.dram_tensor("part", [T, H], bf16, kind="ExternalOutput").ap()
    if DEBUG_DUMP:
        dbg_qkT = nc.dram_tensor("dbg_qkT", [P, 5, T], bf16,
                                 kind="ExternalOutput").ap()
        dbg_vnat = nc.dram_tensor("dbg_vnat", [P, NKC, 160], bf16,
                                  kind="ExternalOutput").ap()
        dbg_attnT = nc.dram_tensor("dbg_attnT", [P, QH, T], bf16,
                                   kind="ExternalOutput").ap()

    with tile.TileContext(nc) as tc:
        with tc.tile_pool(name="keep", bufs=1) as keep, \
             tc.tile_pool(name="hstream", bufs=34) as hsp, \
             tc.tile_pool(name="xfp", bufs=6) as xfp, \
             tc.tile_pool(name="rotp", bufs=3) as rotp, \
             tc.tile_pool(name="t12p", bufs=3) as t12p, \
             tc.tile_pool(name="ep", bufs=20) as ep, \
             tc.tile_pool(name="rdp", bufs=4) as rdp, \
             tc.tile_pool(name="pvsbp", bufs=4) as pvsbp, \
             tc.tile_pool(name="outp", bufs=4) as outp, \
             tc.tile_pool(name="aps", bufs=2, space="PSUM") as aps, \
             tc.tile_pool(name="stps", bufs=2, space="PSUM") as stps, \
             tc.tile_pool(name="wops", bufs=2, space="PSUM") as wops, \
             tc.tile_pool(name="pvps", bufs=2, space="PSUM") as pvps:

            # ---------------- long-lived SBUF ----------------
            wq = keep.tile([P, NHC, 768], bf16, tag="wq_t")
            qkT = keep.tile([P, 5, T], bf16, tag="qkT_t")
            vnat = keep.tile([P, NKC, 160], bf16, tag="vnat_t")
            attnT = keep.tile([P, QH, T], bf16, tag="attnT_t")
            wot = keep.tile([P, QH, H], bf16, tag="wot_t")
            ctk = keep.tile([P, T], bf16, tag="cosk_t")
            stk = keep.tile([P, T], bf16, tag="sink_t")
            mt = keep.tile([P, P], f32, tag="mask_t")

            # hidden chunk 0 first (unblocks the first matmul), then
            # weights and tables.
            hts0 = []
            for h in range(NHC):
                ht = hsp.tile([P, G], bf16, tag="ht", name=f"ht_0_{h}")
                nc.sync.dma_start(ht[:], hidT[h * P:(h + 1) * P, 0:G])
                hts0.append(ht)
            for h in range(NHC):
                nc.sync.dma_start(wq[:, h, :], wq_d[h * P:(h + 1) * P, :])
            nc.sync.dma_start(mt[:], maskd[:])
            nc.sync.dma_start(ctk[:], cosk[:])
            nc.sync.dma_start(stk[:], sink[:])
            nc.vector.memset(vnat[:, :, 128:129], 1.0)
            for hc in range(QH):
                nc.gpsimd.dma_start(wot[:, hc, :], wo_d[:, hc, :])

            # ---- paced fill queue: (est_pe_ns, closure) ----
            fill = collections.deque()

            def pace(budget_ns):
                spent = 0
                while fill and spent < budget_ns:
                    est, fn = fill.popleft()
                    fn()
                    spent += est

            def flush_fill():
                while fill:
                    _, fn = fill.popleft()
                    fn()

            wo_queue = []

            def emit_wo_group():
                tcn, oc = wo_queue.pop(0)
                o_ps = wops.tile([P, G], f32, tag="wo",
                                 name=f"o_{tcn}_{oc}")
                for hc in range(QH):
                    nc.tensor.matmul(
                        o_ps[:], attnT[:, hc, tcn * P:(tcn + 1) * P],
                        wot[:, hc, oc * G:(oc + 1) * G],
                        start=(hc == 0), stop=(hc == QH - 1))
                ob = outp.tile([P, G], bf16, tag="ob",
                               name=f"ob_{tcn}_{oc}")
                if (tcn + oc) % 2 == 0:
                    nc.scalar.copy(ob[:], o_ps[:])
                else:
                    nc.vector.tensor_scalar_add(ob[:], o_ps[:], 0.0)
                nc.gpsimd.dma_start(
                    part[tcn * P:(tcn + 1) * P, oc * G:(oc + 1) * G], ob[:])

            def rope(c, t, xf):
                # qkT[:, c, t*G:+G] = xf*cos + rot(xf)*sin
                cos_t, sin_t = ctk, stk
                sl = slice(t * G, (t + 1) * G)
                rot = rotp.tile([P, G], bf16, tag="rot",
                                name=f"rot_{c}_{t}")
                nc.gpsimd.dma_start(rot[0:HALF, :], xf[HALF:P, :])
                nc.gpsimd.dma_start(rot[HALF:P, :], xf[0:HALF, :])
                t1 = t12p.tile([P, G], f32, tag="t12", name=f"t1_{c}_{t}")
                t2 = t12p.tile([P, G], f32, tag="t12", name=f"t2_{c}_{t}")
                nc.vector.tensor_tensor(t1[:], xf[:], cos_t[:, sl],
                                        mybir.AluOpType.mult)
                nc.vector.tensor_tensor(t2[:], rot[:], sin_t[:, sl],
                                        mybir.AluOpType.mult)
                nc.vector.tensor_tensor(qkT[:, c, sl], t1[:], t2[:],
                                        mybir.AluOpType.add)

            def attn_block(g):
                """Scores+exp for block g; pushes PV/normalize/WO work
                onto the fill queue, paced into the score stream."""
                for head in range(QH):
                    kmax = 4 * (g + 1)
                    e_tiles = []
                    for kc in range(kmax):
                        j = kc - 4 * g
                        W = G if j < 0 else G - P * j
                        qo = g * G + (G - W)
                        stt = stps.tile([P, G], f32, tag="st",
                                        name=f"st_{g}_{head}_{kc}")
                        nc.tensor.matmul(
                            stt[:, 0:W],
                            qkT[:, QH, kc * P:(kc + 1) * P],
                            qkT[:, head, qo:qo + W],
                            start=True, stop=True)
                        if j >= 0:
                            nc.vector.tensor_tensor(
                                stt[:, 0:P], stt[:, 0:P], mt[:],
                                mybir.AluOpType.add)
                        e = ep.tile([P, G], bf16, tag="e",
                                    name=f"e_{g}_{head}_{kc}")
                        nc.scalar.activation(
                            e[:, 0:W], stt[:, 0:W],
                            mybir.ActivationFunctionType.Exp)
                        e_tiles.append((kc, W, e))
                        pace(500)
                    # queue this head's PV sweeps + normalize
                    rd = rdp.tile([P, 4], f32, tag="rd",
                                  name=f"rd_{g}_{head}")
                    for sp in range(2):
                        pvt = [pvps.tile([P, G], f32, tag="pv",
                                         name=f"pv_{g}_{head}_{sp}_{i}")
                               for i in range(2)]

                        def pv_iter(kc, W, e, sp=sp, pvt=pvt, g=g):
                            j = kc - 4 * g
                            for i in range(2):
                                s = 2 * sp + i
                                if j > s or kc > 4 * g + s:
                                    continue
                                eoff = s * P - (G - W)
                                nc.tensor.matmul(
                                    pvt[i][:, 0:129],
                                    e[:, eoff:eoff + P],
                                    vnat[:, kc, 0:129],
                                    start=(kc == 0), stop=(kc == 4 * g + s))

                        for kc, W, e in e_tiles:
                            if kc > 4 * g + 2 * sp + 1:
                                continue
                            fill.append(
                                (160, lambda a=kc, b=W, c=e,
                                 f=pv_iter: f(a, b, c)))

                        def norm_pair(sp=sp, pvt=pvt, rd=rd, g=g,
                                      head=head):
                            for i in range(2):
                                s = 2 * sp + i
                                nc.vector.reciprocal(rd[:, s:s + 1],
                                                     pvt[i][:, 128:129])
                                pvn = pvsbp.tile(
                                    [P, P], bf16, tag="pvn",
                                    name=f"pvn_{g}_{head}_{s}")
                                nc.vector.tensor_scalar(
                                    pvn[:], pvt[i][:, 0:P],
                                    rd[:, s:s + 1], None,
                                    mybir.AluOpType.mult)
                                tsl = (4 * g + s) * P
                                nc.sync.dma_start_transpose(
                                    attnT[:, head, tsl:tsl + P], pvn[:])

                        fill.append((50, norm_pair))
                # WO for this block, after all heads normalized
                wo_queue.extend(
                    [(4 * g + i, oc) for i in range(4) for oc in range(8)])
                for _ in range(32):
                    fill.append((900, emit_wo_group))

            # ---------------- main pipeline ----------------
            for t in range(NG):
                if t == 0:
                    hts = hts0
                else:
                    hts = []
                    for h in range(NHC):
                        ht = hsp.tile([P, G], bf16, tag="ht",
                                      name=f"ht_{t}_{h}")
                        nc.sync.dma_start(
                            ht[:], hidT[h * P:(h + 1) * P,
                                        t * G:(t + 1) * G])
                        hts.append(ht)
                # 6 sequential accumulation groups: q0..q3, k, v^T
                for c in range(6):
                    ps = aps.tile([P, G], f32, tag="aps",
                                  name=f"aps_{t}_{c}")
                    for h in range(NHC):
                        nc.tensor.matmul(
                            ps[:], wq[:, h, c * P:(c + 1) * P], hts[h][:],
                            start=(h == 0), stop=(h == NHC - 1))
                    if c < 5:
                        xf = xfp.tile([P, G], bf16, tag="xf",
                                      name=f"xf_{t}_{c}")
                        nc.scalar.copy(xf[:], ps[:])
                        rope(c, t, xf)
                    else:
                        for s in range(4):
                            xv = pvsbp.tile([P, P], bf16, tag="xv",
                                            name=f"xv_{t}_{s}")
                            nc.scalar.copy(xv[:],
                                           ps[:, s * P:(s + 1) * P])
                            nc.sync.dma_start_transpose(
                                vnat[:, 4 * t + s, 0:P], xv[:])
                attn_block(t)
            flush_fill()
            if DEBUG_DUMP:
                nc.sync.dma_start(dbg_qkT[:], qkT[:])
                nc.sync.dma_start(dbg_vnat[:], vnat[:])
                nc.sync.dma_start(dbg_attnT[:], attnT[:])

    nc.compile()
    return nc


def _rope_tables(positions):
    pos = positions.astype(np.float64)
    inv_freq = 1.0 / (BASE ** (np.arange(HALF, dtype=np.float64) / HALF))
    freqs = pos[:, None] * inv_freq[None, :]          # [T, 64]
    cos = np.cos(freqs)
    sin = np.sin(freqs)
    cosT = np.concatenate([cos, cos], axis=1).T       # [128, T]
    sinT = np.concatenate([-sin, sin], axis=1).T      # sign folded
    return cosT, sinT


def kernel(positions, hidden_states, wqkv, wo):
    global _COMPILED
    if _COMPILED is None:
        _COMPILED = _build()
    nc = _COMPILED

    cosT, sinT = _rope_tables(positions)
    cosk = np.ascontiguousarray(cosT).astype(bfloat16)
    sink = np.ascontiguousarray(sinT).astype(bfloat16)

    hidT = np.ascontiguousarray(np.asarray(hidden_states).T).astype(bfloat16)

    # causal triangle for the diagonal 128x128 sub-block, [k, q] layout
    kl = np.arange(P)[:, None]
    ql = np.arange(P)[None, :]
    maskd = np.where(kl <= ql, 0.0, NEG).astype(np.float32)

    wqkv = np.asarray(wqkv)
    wo = np.asarray(wo)
    in_maps = []
    for r in range(NCORES):
        qc = slice(r * QCOLS, (r + 1) * QCOLS)
        kc = slice(NH * HD + r * HD, NH * HD + (r + 1) * HD)
        vc = slice((NH + NKV) * HD + r * HD, (NH + NKV) * HD + (r + 1) * HD)
        wq_s = np.ascontiguousarray(np.concatenate(
            [wqkv[:, qc], wqkv[:, kc] * (HD ** -0.5), wqkv[:, vc]],
            axis=1)).astype(bfloat16)
        wo_r = np.ascontiguousarray(
            wo[qc, :].reshape(QH, P, H).transpose(1, 0, 2)).astype(bfloat16)
        in_maps.append({
            "hidT": hidT, "wq_d": wq_s, "wo_d": wo_r,
            "cosk": cosk, "sink": sink, "maskd": maskd,
        })

    global _LAST_IN_MAPS
    _LAST_IN_MAPS = in_maps
    res = run_bass_kernel_spmd(nc, in_maps, list(range(NCORES)))
    out = res.results[0]["part"].astype(np.float64)
    for r in range(1, NCORES):
        out += res.results[r]["part"].astype(np.float64)
    return out.astype(np.float32)


# revision 21
# speedup vs baseline: 1.3685x; 1.3685x over previous
"""InternLM3 self-attention (prefill, GQA, RoPE) on 8 Trainium2 cores.

Tensor-parallel over heads: core r owns q heads 4r..4r+3 and kv head r
(wqkv column shards, wo row shards).  Each core computes its partial
output projection in bf16; the 8 partials are summed on the host.

v2.2 design (baseline fp32r v1 was ~640-700us):
  - every matmul in bf16 with 512-col moving operands (LDWEIGHTS mostly
    hidden behind the previous matmul; 1 cycle/row; fp32 PSUM).
  - softmax denominator for free: v rows carry an appended ones column,
    so the PV matmul accumulates sum(e) in PSUM column 128.
  - PV computed transposed (out [q, hd]) so 1/d normalization is a
    per-partition tensor_scalar with [128,1] reciprocals; the result
    moves into WO layout with a DMA-XBAR transpose (no PE).
  - v projected as v^T like q/k, then moved to natural [tok, hd] layout
    with DMA-XBAR transposes (no PE, no PSUM).
  - causal trimming at 128 granularity (diagonal 512-blocks stepped).
  - RoPE rotate-half via two partition-offset SBUF DMAs (no PE).
  - interleaved emission: the exp-gated score stream of each head is
    padded with a paced fill queue carrying the previous head's PV /
    normalize work and the previous block's WO groups, so the PE never
    waits on the scalar engine's exp stream.
  - PSUM discipline: concurrently-accumulating matmul groups sit in
    separate banks (start=True clears the whole bank's has_written).
"""

import collections

import numpy as np
from ml_dtypes import bfloat16

import concourse.bass as bass
import concourse.bacc as bacc
import concourse.mybir as mybir
import concourse.tile as tile
from concourse.bass_utils import run_bass_kernel_spmd

T = 2048
H = 4096
NH = 32
NKV = 8
HD = 128
HALF = HD // 2
BASE = 1000000.0
NCORES = 8
QH = NH // NCORES            # 4 q heads per core
QCOLS = QH * HD              # 512
NEG = -1e30

P = 128
G = 512                      # token chunk = attention q block
NG = T // G                  # 4
NHC = H // P                 # 32 contraction chunks
NKC = T // P                 # 16 k chunks of 128

f32 = mybir.dt.float32
bf16 = mybir.dt.bfloat16

_COMPILED = None
DEBUG_DUMP = False


def _build():
    nc = bacc.Bacc("TRN2", target_bir_lowering=False, debug=False,
                   num_devices=NCORES)

    hidT = nc.dram_tensor("hidT", [H, T], bf16, kind="ExternalInput").ap()
    wq_d = nc.dram_tensor("wq_d", [H, 768], bf16, kind="ExternalInput").ap()
    wo_d = nc.dram_tensor("wo_d", [P, QH, H], bf16,
                          kind="ExternalInput").ap()
    cosk = nc.dram_tensor("cosk", [P, T], bf16, kind="ExternalInput").ap()
    sink = nc.dram_tensor("sink", [P, T], bf16, kind="ExternalInput").ap()
    maskd = nc.dram_tensor("maskd", [P, P], f32, kind="ExternalInput").ap()
    identd = nc.dram_tensor("identd", [P, P], bf16,
                            kind="ExternalInput").ap()
    part = nc.dram_tensor("part", [T, H], bf16, kind="ExternalOutput").ap()
    if DEBUG_DUMP:
        dbg_qkT = nc.dram_tensor("dbg_qkT", [P, 5, T], bf16,
                                 kind="ExternalOutput").ap()
        dbg_vnat = nc.dram_tensor("dbg_vnat", [P, NKC, 160], bf16,
                                  kind="ExternalOutput").ap()
        dbg_attnT = nc.dram_tensor("dbg_attnT", [P, QH, T], bf16,
                                   kind="ExternalOutput").ap()

    with tile.TileContext(nc) as tc:
        with tc.tile_pool(name="keep", bufs=1) as keep, \
             tc.tile_pool(name="hstream", bufs=34) as hsp, \
             tc.tile_pool(name="xfp", bufs=6) as xfp, \
             tc.tile_pool(name="rotp", bufs=3) as rotp, \
             tc.tile_pool(name="t12p", bufs=3) as t12p, \
             tc.tile_pool(name="ep", bufs=20) as ep, \
             tc.tile_pool(name="rdp", bufs=4) as rdp, \
             tc.tile_pool(name="pvsbp", bufs=4) as pvsbp, \
             tc.tile_pool(name="outp", bufs=4) as outp, \
             tc.tile_pool(name="aps", bufs=2, space="PSUM") as aps, \
             tc.tile_pool(name="stps", bufs=2, space="PSUM") as stps, \
             tc.tile_pool(name="wops", bufs=2, space="PSUM") as wops, \
             tc.tile_pool(name="pvps", bufs=2, space="PSUM") as pvps:

            # ---------------- long-lived SBUF ----------------
            wq = keep.tile([P, NHC, 768], bf16, tag="wq_t")
            qkT = keep.tile([P, 5, T], bf16, tag="qkT_t")
            vnat = keep.tile([P, NKC, 160], bf16, tag="vnat_t")
            attnT = keep.tile([P, QH, T], bf16, tag="attnT_t")
            wot = keep.tile([P, QH, H], bf16, tag="wot_t")
            ctk = keep.tile([P, T], bf16, tag="cosk_t")
            stk = keep.tile([P, T], bf16, tag="sink_t")
            mt = keep.tile([P, P], f32, tag="mask_t")
            idt = keep.tile([P, P], bf16, tag="ident_t")

            # hidden chunk 0 first (unblocks the first matmul), then
            # weights and tables.
            hts0 = []
            for h in range(NHC):
                ht = hsp.tile([P, G], bf16, tag="ht", name=f"ht_0_{h}")
                nc.sync.dma_start(ht[:], hidT[h * P:(h + 1) * P, 0:G])
                hts0.append(ht)
            for h in range(NHC):
                nc.sync.dma_start(wq[:, h, :], wq_d[h * P:(h + 1) * P, :])
            nc.sync.dma_start(mt[:], maskd[:])
            nc.sync.dma_start(idt[:], identd[:])
            nc.sync.dma_start(ctk[:], cosk[:])
            nc.sync.dma_start(stk[:], sink[:])
            nc.vector.memset(vnat[:, :, 128:129], 1.0)
            for hc in range(QH):
                nc.gpsimd.dma_start(wot[:, hc, :], wo_d[:, hc, :])

            # ---- paced fill queue: (est_pe_ns, closure) ----
            fill = collections.deque()

            def pace(budget_ns):
                spent = 0
                while fill and spent < budget_ns:
                    est, fn = fill.popleft()
                    fn()
                    spent += est

            def flush_fill():
                while fill:
                    _, fn = fill.popleft()
                    fn()

            wo_queue = []

            def emit_wo_group():
                tcn, oc = wo_queue.pop(0)
                o_ps = wops.tile([P, G], f32, tag="wo",
                                 name=f"o_{tcn}_{oc}")
                for hc in range(QH):
                    nc.tensor.matmul(
                        o_ps[:], attnT[:, hc, tcn * P:(tcn + 1) * P],
                        wot[:, hc, oc * G:(oc + 1) * G],
                        start=(hc == 0), stop=(hc == QH - 1))
                ob = outp.tile([P, G], bf16, tag="ob",
                               name=f"ob_{tcn}_{oc}")
                nc.scalar.copy(ob[:], o_ps[:])
                nc.gpsimd.dma_start(
                    part[tcn * P:(tcn + 1) * P, oc * G:(oc + 1) * G], ob[:])

            def rope(c, t, xf):
                # qkT[:, c, t*G:+G] = xf*cos + rot(xf)*sin
                cos_t, sin_t = ctk, stk
                sl = slice(t * G, (t + 1) * G)
                rot = rotp.tile([P, G], bf16, tag="rot",
                                name=f"rot_{c}_{t}")
                nc.gpsimd.dma_start(rot[0:HALF, :], xf[HALF:P, :])
                nc.gpsimd.dma_start(rot[HALF:P, :], xf[0:HALF, :])
                t1 = t12p.tile([P, G], f32, tag="t12", name=f"t1_{c}_{t}")
                t2 = t12p.tile([P, G], f32, tag="t12", name=f"t2_{c}_{t}")
                nc.vector.tensor_tensor(t1[:], xf[:], cos_t[:, sl],
                                        mybir.AluOpType.mult)
                nc.vector.tensor_tensor(t2[:], rot[:], sin_t[:, sl],
                                        mybir.AluOpType.mult)
                nc.vector.tensor_tensor(qkT[:, c, sl], t1[:], t2[:],
                                        mybir.AluOpType.add)

            def attn_block(g):
                """Scores+exp for block g; pushes PV/normalize/WO work
                onto the fill queue, paced into the score stream."""
                for head in range(QH):
                    kmax = 4 * (g + 1)
                    e_tiles = []
                    for kc in range(kmax):
                        j = kc - 4 * g
                        W = G if j < 0 else G - P * j
                        qo = g * G + (G - W)
                        stt = stps.tile([P, G], f32, tag="st",
                                        name=f"st_{g}_{head}_{kc}")
                        nc.tensor.matmul(
                            stt[:, 0:W],
                            qkT[:, QH, kc * P:(kc + 1) * P],
                            qkT[:, head, qo:qo + W],
                            start=True, stop=True)
                        if j >= 0:
                            nc.vector.tensor_tensor(
                                stt[:, 0:P], stt[:, 0:P], mt[:],
                                mybir.AluOpType.add)
                        e = ep.tile([P, G], bf16, tag="e",
                                    name=f"e_{g}_{head}_{kc}")
                        nc.scalar.activation(
                            e[:, 0:W], stt[:, 0:W],
                            mybir.ActivationFunctionType.Exp)
                        e_tiles.append((kc, W, e))
                        pace(500)
                    # queue this head's PV sweeps + normalize
                    rd = rdp.tile([P, 4], f32, tag="rd",
                                  name=f"rd_{g}_{head}")
                    for sp in range(2):
                        pvt = [pvps.tile([P, G], f32, tag="pv",
                                         name=f"pv_{g}_{head}_{sp}_{i}")
                               for i in range(2)]

                        def pv_iter(kc, W, e, sp=sp, pvt=pvt, g=g):
                            j = kc - 4 * g
                            for i in range(2):
                                s = 2 * sp + i
                                if j > s or kc > 4 * g + s:
                                    continue
                                eoff = s * P - (G - W)
                                nc.tensor.matmul(
                                    pvt[i][:, 0:129],
                                    e[:, eoff:eoff + P],
                                    vnat[:, kc, 0:129],
                                    start=(kc == 0), stop=(kc == 4 * g + s))

                        for kc, W, e in e_tiles:
                            if kc > 4 * g + 2 * sp + 1:
                                continue
                            fill.append(
                                (160, lambda a=kc, b=W, c=e,
                                 f=pv_iter: f(a, b, c)))

                        def norm_pair(sp=sp, pvt=pvt, rd=rd, g=g,
                                      head=head):
                            for i in range(2):
                                s = 2 * sp + i
                                nc.vector.reciprocal(rd[:, s:s + 1],
                                                     pvt[i][:, 128:129])
                                pvn = pvsbp.tile(
                                    [P, P], bf16, tag="pvn",
                                    name=f"pvn_{g}_{head}_{s}")
                                nc.vector.tensor_scalar(
                                    pvn[:], pvt[i][:, 0:P],
                                    rd[:, s:s + 1], None,
                                    mybir.AluOpType.mult)
                                tp = pvt[i][:, 256:320].bitcast(bf16)
                                nc.tensor.transpose(tp, pvn[:], idt[:])
                                tsl = (4 * g + s) * P
                                nc.scalar.copy(
                                    attnT[:, head, tsl:tsl + P], tp)

                        fill.append((50, norm_pair))
                # WO for this block, after all heads normalized
                wo_queue.extend(
                    [(4 * g + i, oc) for i in range(4) for oc in range(8)])
                for _ in range(32):
                    fill.append((900, emit_wo_group))

            # ---------------- main pipeline ----------------
            nhts = hts0
            for t in range(NG):
                hts = nhts
                # 6 sequential accumulation groups: q0..q3, k, v^T
                for c in range(6):
                    ps = aps.tile([P, G], f32, tag="aps",
                                  name=f"aps_{t}_{c}")
                    for h in range(NHC):
                        nc.tensor.matmul(
                            ps[:], wq[:, h, c * P:(c + 1) * P], hts[h][:],
                            start=(h == 0), stop=(h == NHC - 1))
                    if c < 5:
                        xf = xfp.tile([P, G], bf16, tag="xf",
                                      name=f"xf_{t}_{c}")
                        nc.scalar.copy(xf[:], ps[:])
                        rope(c, t, xf)
                    else:
                        for s in range(4):
                            xv = pvsbp.tile([P, P], bf16, tag="xv",
                                            name=f"xv_{t}_{s}")
                            nc.scalar.copy(xv[:],
                                           ps[:, s * P:(s + 1) * P])
                            nc.sync.dma_start_transpose(
                                vnat[:, 4 * t + s, 0:P], xv[:])
                if t + 1 < NG:
                    nhts = []
                    for h in range(NHC):
                        ht = hsp.tile([P, G], bf16, tag="ht",
                                      name=f"ht_{t + 1}_{h}")
                        nc.sync.dma_start(
                            ht[:], hidT[h * P:(h + 1) * P,
                                        (t + 1) * G:(t + 2) * G])
                        nhts.append(ht)
                attn_block(t)
            flush_fill()
            if DEBUG_DUMP:
                nc.sync.dma_start(dbg_qkT[:], qkT[:])
                nc.sync.dma_start(dbg_vnat[:], vnat[:])
                nc.sync.dma_start(dbg_attnT[:], attnT[:])

    nc.compile()
    return nc


def _rope_tables(positions):
    pos = positions.astype(np.float64)
    inv_freq = 1.0 / (BASE ** (np.arange(HALF, dtype=np.float64) / HALF))
    freqs = pos[:, None] * inv_freq[None, :]          # [T, 64]
    cos = np.cos(freqs)
    sin = np.sin(freqs)
    cosT = np.concatenate([cos, cos], axis=1).T       # [128, T]
    sinT = np.concatenate([-sin, sin], axis=1).T      # sign folded
    return cosT, sinT


def kernel(positions, hidden_states, wqkv, wo):
    global _COMPILED
    if _COMPILED is None:
        _COMPILED = _build()
    nc = _COMPILED

    cosT, sinT = _rope_tables(positions)
    cosk = np.ascontiguousarray(cosT).astype(bfloat16)
    sink = np.ascontiguousarray(sinT).astype(bfloat16)

    hidT = np.ascontiguousarray(np.asarray(hidden_states).T).astype(bfloat16)

    # causal triangle for the diagonal 128x128 sub-block, [k, q] layout
    kl = np.arange(P)[:, None]
    ql = np.arange(P)[None, :]
    maskd = np.where(kl <= ql, 0.0, NEG).astype(np.float32)

    wqkv = np.asarray(wqkv)
    wo = np.asarray(wo)
    in_maps = []
    for r in range(NCORES):
        qc = slice(r * QCOLS, (r + 1) * QCOLS)
        kc = slice(NH * HD + r * HD, NH * HD + (r + 1) * HD)
        vc = slice((NH + NKV) * HD + r * HD, (NH + NKV) * HD + (r + 1) * HD)
        wq_s = np.ascontiguousarray(np.concatenate(
            [wqkv[:, qc], wqkv[:, kc] * (HD ** -0.5), wqkv[:, vc]],
            axis=1)).astype(bfloat16)
        wo_r = np.ascontiguousarray(
            wo[qc, :].reshape(QH, P, H).transpose(1, 0, 2)).astype(bfloat16)
        in_maps.append({
            "hidT": hidT, "wq_d": wq_s, "wo_d": wo_r,
            "cosk": cosk, "sink": sink, "maskd": maskd,
            "identd": np.eye(P, dtype=np.float32).astype(bfloat16),
        })

    global _LAST_IN_MAPS
    _LAST_IN_MAPS = in_maps
    res = run_bass_kernel_spmd(nc, in_maps, list(range(NCORES)))
    out = res.results[0]["part"].astype(np.float64)
    for r in range(1, NCORES):
        out += res.results[r]["part"].astype(np.float64)
    return out.astype(np.float32)


# revision 22
# speedup vs baseline: 1.4798x; 1.0813x over previous
"""InternLM3 self-attention (prefill, GQA, RoPE) on 8 Trainium2 cores.

Tensor-parallel over heads: core r owns q heads 4r..4r+3 and kv head r
(wqkv column shards, wo row shards).  Each core computes its partial
output projection in bf16; the 8 partials are summed on the host.

v2.2 design (baseline fp32r v1 was ~640-700us):
  - every matmul in bf16 with 512-col moving operands (LDWEIGHTS mostly
    hidden behind the previous matmul; 1 cycle/row; fp32 PSUM).
  - softmax denominator for free: v rows carry an appended ones column,
    so the PV matmul accumulates sum(e) in PSUM column 128.
  - PV computed transposed (out [q, hd]) so 1/d normalization is a
    per-partition tensor_scalar with [128,1] reciprocals; the result
    moves into WO layout with a DMA-XBAR transpose (no PE).
  - v projected as v^T like q/k, then moved to natural [tok, hd] layout
    with DMA-XBAR transposes (no PE, no PSUM).
  - causal trimming at 128 granularity (diagonal 512-blocks stepped).
  - RoPE rotate-half via two partition-offset SBUF DMAs (no PE).
  - interleaved emission: the exp-gated score stream of each head is
    padded with a paced fill queue carrying the previous head's PV /
    normalize work and the previous block's WO groups, so the PE never
    waits on the scalar engine's exp stream.
  - PSUM discipline: concurrently-accumulating matmul groups sit in
    separate banks (start=True clears the whole bank's has_written).
"""

import collections

import numpy as np
from ml_dtypes import bfloat16

import concourse.bass as bass
import concourse.bacc as bacc
import concourse.mybir as mybir
import concourse.tile as tile
from concourse.bass_utils import run_bass_kernel_spmd

T = 2048
H = 4096
NH = 32
NKV = 8
HD = 128
HALF = HD // 2
BASE = 1000000.0
NCORES = 8
QH = NH // NCORES            # 4 q heads per core
QCOLS = QH * HD              # 512
NEG = -1e30

P = 128
G = 512                      # token chunk = attention q block
NG = T // G                  # 4
NHC = H // P                 # 32 contraction chunks
NKC = T // P                 # 16 k chunks of 128

f32 = mybir.dt.float32
bf16 = mybir.dt.bfloat16

_COMPILED = None
DEBUG_DUMP = False


def _build():
    nc = bacc.Bacc("TRN2", target_bir_lowering=False, debug=False,
                   num_devices=NCORES)

    hidT = nc.dram_tensor("hidT", [H, T], bf16, kind="ExternalInput").ap()
    wq_d = nc.dram_tensor("wq_d", [H, 768], bf16, kind="ExternalInput").ap()
    wo_d = nc.dram_tensor("wo_d", [P, QH, H], bf16,
                          kind="ExternalInput").ap()
    cosk = nc.dram_tensor("cosk", [P, T], bf16, kind="ExternalInput").ap()
    sink = nc.dram_tensor("sink", [P, T], bf16, kind="ExternalInput").ap()
    maskd = nc.dram_tensor("maskd", [P, P], f32, kind="ExternalInput").ap()
    identd = nc.dram_tensor("identd", [P, P], bf16,
                            kind="ExternalInput").ap()
    part = nc.dram_tensor("part", [T, H], bf16, kind="ExternalOutput").ap()
    if DEBUG_DUMP:
        dbg_qkT = nc.dram_tensor("dbg_qkT", [P, 5, T], bf16,
                                 kind="ExternalOutput").ap()
        dbg_vnat = nc.dram_tensor("dbg_vnat", [P, NKC, 160], bf16,
                                  kind="ExternalOutput").ap()
        dbg_attnT = nc.dram_tensor("dbg_attnT", [P, QH, T], bf16,
                                   kind="ExternalOutput").ap()

    with tile.TileContext(nc) as tc:
        with tc.tile_pool(name="keep", bufs=1) as keep, \
             tc.tile_pool(name="hstream", bufs=34) as hsp, \
             tc.tile_pool(name="xfp", bufs=6) as xfp, \
             tc.tile_pool(name="rotp", bufs=3) as rotp, \
             tc.tile_pool(name="t12p", bufs=3) as t12p, \
             tc.tile_pool(name="ep", bufs=20) as ep, \
             tc.tile_pool(name="rdp", bufs=4) as rdp, \
             tc.tile_pool(name="pvsbp", bufs=4) as pvsbp, \
             tc.tile_pool(name="outp", bufs=4) as outp, \
             tc.tile_pool(name="aps", bufs=2, space="PSUM") as aps, \
             tc.tile_pool(name="stps", bufs=2, space="PSUM") as stps, \
             tc.tile_pool(name="wops", bufs=2, space="PSUM") as wops, \
             tc.tile_pool(name="pvps", bufs=2, space="PSUM") as pvps:

            # ---------------- long-lived SBUF ----------------
            wq = keep.tile([P, NHC, 768], bf16, tag="wq_t")
            qkT = keep.tile([P, 5, T], bf16, tag="qkT_t")
            vnat = keep.tile([P, NKC, 160], bf16, tag="vnat_t")
            attnT = keep.tile([P, QH, T], bf16, tag="attnT_t")
            wot = keep.tile([P, QH, H], bf16, tag="wot_t")
            ctk = keep.tile([P, T], bf16, tag="cosk_t")
            stk = keep.tile([P, T], bf16, tag="sink_t")
            mt = keep.tile([P, P], f32, tag="mask_t")
            idt = keep.tile([P, P], bf16, tag="ident_t")

            # hidden chunk 0 first (unblocks the first matmul), then
            # weights and tables.
            hts0 = []
            for h in range(NHC):
                ht = hsp.tile([P, G], bf16, tag="ht", name=f"ht_0_{h}")
                nc.sync.dma_start(ht[:], hidT[h * P:(h + 1) * P, 0:G])
                hts0.append(ht)
            for h in range(NHC):
                nc.sync.dma_start(wq[:, h, :], wq_d[h * P:(h + 1) * P, :])
            nc.sync.dma_start(mt[:], maskd[:])
            nc.sync.dma_start(idt[:], identd[:])
            nc.sync.dma_start(ctk[:], cosk[:])
            nc.sync.dma_start(stk[:], sink[:])
            nc.vector.memset(vnat[:, :, 128:129], 1.0)
            for hc in range(QH):
                nc.gpsimd.dma_start(wot[:, hc, :], wo_d[:, hc, :])

            # ---- paced fill queue: (est_pe_ns, closure) ----
            fill = collections.deque()

            def pace(budget_ns):
                spent = 0
                while fill and spent < budget_ns:
                    est, fn = fill.popleft()
                    fn()
                    spent += est

            def flush_fill():
                while fill:
                    _, fn = fill.popleft()
                    fn()

            wo_queue = []

            def emit_wo_group():
                tcn, oc = wo_queue.pop(0)
                o_ps = wops.tile([P, G], f32, tag="wo",
                                 name=f"o_{tcn}_{oc}")
                for hc in range(QH):
                    nc.tensor.matmul(
                        o_ps[:], attnT[:, hc, tcn * P:(tcn + 1) * P],
                        wot[:, hc, oc * G:(oc + 1) * G],
                        start=(hc == 0), stop=(hc == QH - 1))
                ob = outp.tile([P, G], bf16, tag="ob",
                               name=f"ob_{tcn}_{oc}")
                nc.scalar.copy(ob[:], o_ps[:])
                nc.gpsimd.dma_start(
                    part[tcn * P:(tcn + 1) * P, oc * G:(oc + 1) * G], ob[:])

            def rope(c, t, xf):
                # qkT[:, c, t*G:+G] = xf*cos + rot(xf)*sin
                cos_t, sin_t = ctk, stk
                sl = slice(t * G, (t + 1) * G)
                rot = rotp.tile([P, G], bf16, tag="rot",
                                name=f"rot_{c}_{t}")
                nc.gpsimd.dma_start(rot[0:HALF, :], xf[HALF:P, :])
                nc.gpsimd.dma_start(rot[HALF:P, :], xf[0:HALF, :])
                t1 = t12p.tile([P, G], f32, tag="t12", name=f"t1_{c}_{t}")
                t2 = t12p.tile([P, G], f32, tag="t12", name=f"t2_{c}_{t}")
                nc.vector.tensor_tensor(t1[:], xf[:], cos_t[:, sl],
                                        mybir.AluOpType.mult)
                nc.vector.tensor_tensor(t2[:], rot[:], sin_t[:, sl],
                                        mybir.AluOpType.mult)
                nc.vector.tensor_tensor(qkT[:, c, sl], t1[:], t2[:],
                                        mybir.AluOpType.add)

            def attn_block(g):
                """Scores+exp for block g; pushes PV/normalize/WO work
                onto the fill queue, paced into the score stream."""
                for head in range(QH):
                    kmax = 4 * (g + 1)
                    e_tiles = []
                    for kc in range(kmax):
                        j = kc - 4 * g
                        W = G if j < 0 else G - P * j
                        qo = g * G + (G - W)
                        stt = stps.tile([P, G], f32, tag="st",
                                        name=f"st_{g}_{head}_{kc}")
                        nc.tensor.matmul(
                            stt[:, 0:W],
                            qkT[:, QH, kc * P:(kc + 1) * P],
                            qkT[:, head, qo:qo + W],
                            start=True, stop=True)
                        if j >= 0:
                            nc.vector.tensor_tensor(
                                stt[:, 0:P], stt[:, 0:P], mt[:],
                                mybir.AluOpType.add)
                        e = ep.tile([P, G], bf16, tag="e",
                                    name=f"e_{g}_{head}_{kc}")
                        nc.scalar.activation(
                            e[:, 0:W], stt[:, 0:W],
                            mybir.ActivationFunctionType.Exp)
                        e_tiles.append((kc, W, e))
                        pace(500)
                    # queue this head's PV sweeps + normalize
                    rd = rdp.tile([P, 4], f32, tag="rd",
                                  name=f"rd_{g}_{head}")
                    for sp in range(2):
                        pvt = [pvps.tile([P, G], f32, tag="pv",
                                         name=f"pv_{g}_{head}_{sp}_{i}")
                               for i in range(2)]

                        def pv_iter(kc, W, e, sp=sp, pvt=pvt, g=g):
                            j = kc - 4 * g
                            for i in range(2):
                                s = 2 * sp + i
                                if j > s or kc > 4 * g + s:
                                    continue
                                eoff = s * P - (G - W)
                                nc.tensor.matmul(
                                    pvt[i][:, 0:129],
                                    e[:, eoff:eoff + P],
                                    vnat[:, kc, 0:129],
                                    start=(kc == 0), stop=(kc == 4 * g + s))

                        for kc, W, e in e_tiles:
                            if kc > 4 * g + 2 * sp + 1:
                                continue
                            fill.append(
                                (160, lambda a=kc, b=W, c=e,
                                 f=pv_iter: f(a, b, c)))

                        def norm_pair(sp=sp, pvt=pvt, rd=rd, g=g,
                                      head=head):
                            for i in range(2):
                                s = 2 * sp + i
                                nc.vector.reciprocal(rd[:, s:s + 1],
                                                     pvt[i][:, 128:129])
                                pvn = pvsbp.tile(
                                    [P, P], bf16, tag="pvn",
                                    name=f"pvn_{g}_{head}_{s}")
                                nc.vector.tensor_scalar(
                                    pvn[:], pvt[i][:, 0:P],
                                    rd[:, s:s + 1], None,
                                    mybir.AluOpType.mult)
                                tp = pvt[i][:, 256:320].bitcast(bf16)
                                nc.tensor.transpose(tp, pvn[:], idt[:])
                                tsl = (4 * g + s) * P
                                nc.vector.tensor_scalar_add(
                                    attnT[:, head, tsl:tsl + P], tp, 0.0)

                        fill.append((50, norm_pair))
                # WO for this block, after all heads normalized
                wo_queue.extend(
                    [(4 * g + i, oc) for i in range(4) for oc in range(8)])
                for _ in range(32):
                    fill.append((900, emit_wo_group))

            # ---------------- main pipeline ----------------
            nhts = hts0
            for t in range(NG):
                hts = nhts
                # 6 sequential accumulation groups, v^T and k first so
                # the vnat transposes and k-rope land before attention
                for c in (5, 4, 0, 1, 2, 3):
                    ps = aps.tile([P, G], f32, tag="aps",
                                  name=f"aps_{t}_{c}")
                    for h in range(NHC):
                        nc.tensor.matmul(
                            ps[:], wq[:, h, c * P:(c + 1) * P], hts[h][:],
                            start=(h == 0), stop=(h == NHC - 1))
                    if c < 5:
                        xf = xfp.tile([P, G], bf16, tag="xf",
                                      name=f"xf_{t}_{c}")
                        nc.scalar.copy(xf[:], ps[:])
                        rope(c, t, xf)
                    else:
                        for s in range(4):
                            xv = pvsbp.tile([P, P], bf16, tag="xv",
                                            name=f"xv_{t}_{s}")
                            nc.scalar.copy(xv[:],
                                           ps[:, s * P:(s + 1) * P])
                            nc.sync.dma_start_transpose(
                                vnat[:, 4 * t + s, 0:P], xv[:])
                    if c == 4 and t + 1 < NG:
                        nhts = []
                        for h in range(NHC):
                            ht = hsp.tile([P, G], bf16, tag="ht",
                                          name=f"ht_{t + 1}_{h}")
                            nc.sync.dma_start(
                                ht[:], hidT[h * P:(h + 1) * P,
                                            (t + 1) * G:(t + 2) * G])
                            nhts.append(ht)
                attn_block(t)
            flush_fill()
            if DEBUG_DUMP:
                nc.sync.dma_start(dbg_qkT[:], qkT[:])
                nc.sync.dma_start(dbg_vnat[:], vnat[:])
                nc.sync.dma_start(dbg_attnT[:], attnT[:])

    nc.compile()
    return nc


def _rope_tables(positions):
    pos = positions.astype(np.float64)
    inv_freq = 1.0 / (BASE ** (np.arange(HALF, dtype=np.float64) / HALF))
    freqs = pos[:, None] * inv_freq[None, :]          # [T, 64]
    cos = np.cos(freqs)
    sin = np.sin(freqs)
    cosT = np.concatenate([cos, cos], axis=1).T       # [128, T]
    sinT = np.concatenate([-sin, sin], axis=1).T      # sign folded
    return cosT, sinT


def kernel(positions, hidden_states, wqkv, wo):
    global _COMPILED
    if _COMPILED is None:
        _COMPILED = _build()
    nc = _COMPILED

    cosT, sinT = _rope_tables(positions)
    cosk = np.ascontiguousarray(cosT).astype(bfloat16)
    sink = np.ascontiguousarray(sinT).astype(bfloat16)

    hidT = np.ascontiguousarray(np.asarray(hidden_states).T).astype(bfloat16)

    # causal triangle for the diagonal 128x128 sub-block, [k, q] layout
    kl = np.arange(P)[:, None]
    ql = np.arange(P)[None, :]
    maskd = np.where(kl <= ql, 0.0, NEG).astype(np.float32)

    wqkv = np.asarray(wqkv)
    wo = np.asarray(wo)
    in_maps = []
    for r in range(NCORES):
        qc = slice(r * QCOLS, (r + 1) * QCOLS)
        kc = slice(NH * HD + r * HD, NH * HD + (r + 1) * HD)
        vc = slice((NH + NKV) * HD + r * HD, (NH + NKV) * HD + (r + 1) * HD)
        wq_s = np.ascontiguousarray(np.concatenate(
            [wqkv[:, qc], wqkv[:, kc] * (HD ** -0.5), wqkv[:, vc]],
            axis=1)).astype(bfloat16)
        wo_r = np.ascontiguousarray(
            wo[qc, :].reshape(QH, P, H).transpose(1, 0, 2)).astype(bfloat16)
        in_maps.append({
            "hidT": hidT, "wq_d": wq_s, "wo_d": wo_r,
            "cosk": cosk, "sink": sink, "maskd": maskd,
            "identd": np.eye(P, dtype=np.float32).astype(bfloat16),
        })

    global _LAST_IN_MAPS
    _LAST_IN_MAPS = in_maps
    res = run_bass_kernel_spmd(nc, in_maps, list(range(NCORES)))
    out = res.results[0]["part"].astype(np.float64)
    for r in range(1, NCORES):
        out += res.results[r]["part"].astype(np.float64)
    return out.astype(np.float32)
